# revision 9
# baseline (speedup 1.0000x reference)
"""EnhancedGAT (3-layer GAT + WeightedSumAndMax readout + MLP) on 8 TRN2
NeuronCores.

Sharding: nodes/graphs partitioned 8 ways (graph-aligned); GAT/MLP weights
replicated. Per layer: node-sharded dense projection -> AllGather of a compact
bf16 node table [z|el|er] -> edge phase with dma_gather row gathers (by src)
and selection-matrix PSUM aggregation (by dst). Selection matrices S / S_T are
host-precomputed constants (edge structure is static per call).

Self-contained: hardcodes the problem geometry; host side only reorders /
packs indices and casts weights (no FLOPs of the model are done on host).
"""
import math
import numpy as np
import ml_dtypes

bf16 = ml_dtypes.bfloat16

NC = 8           # cores
PB = 128         # partition/block size
TMAX = 8         # max tiles per dma_gather op (num_idxs <= 1024)
SPLIT = 32768    # int16 index range split for the gather table
NQ = 4           # SWDGE queues
EPS = 1e-5

_CACHE = {}


# --------------------------------------------------------------------------
# host-side preprocessing
# --------------------------------------------------------------------------

def _ceil(a, b):
    return -(-a // b)


def _pack_idx16(lst):
    """gather list -> [128, n/16] int16 tile (16-wrapped, replicated x8)."""
    n = len(lst)
    assert n % 16 == 0
    a = np.asarray(lst, np.int16).reshape(n // 16, 16).T.copy()  # [16, n/16]
    return np.tile(a, (8, 1))


def _prep(inputs):
    h = np.asarray(inputs["h"], np.float32)
    src = np.asarray(inputs["src"]).astype(np.int64).ravel()
    dst = np.asarray(inputs["dst"]).astype(np.int64).ravel()
    gid = np.asarray(inputs["graph_ids"]).astype(np.int64).ravel()

    N, F = h.shape
    E = src.shape[0]
    G = int(gid.max()) + 1
    K, H = np.asarray(inputs["al0"]).shape
    KH = K * H

    assert N % NC == 0 and G % NC == 0 and F == PB and KH == 2 * PB
    NSH = N // NC
    GSH = G // NC
    NBLK = _ceil(NSH, PB)
    XR = NBLK * PB           # padded x rows per shard
    WROW = 3 * KH // 2       # 384 bf16 = 768B table row: [z 256|el 4|er 4|pad]
    assert (WROW * 2) % 256 == 0

    # graph boundaries must align with node shards
    bounds = np.searchsorted(gid, np.arange(G))
    assert all(bounds[c * GSH] == c * NSH for c in range(NC))

    # ---- edge partition: core = dst // NSH, block = local dst // 128
    core_of = dst // NSH
    dl = dst - core_of * NSH
    blk = dl // PB
    dib = (dl % PB).astype(np.int32)          # dst-in-block column
    half = (src >= SPLIT).astype(np.int64)
    key = (core_of * NBLK + blk) * 2 + half
    order = np.argsort(key, kind="stable")
    cnt = np.bincount(key, minlength=NC * NBLK * 2).reshape(NC, NBLK, 2)
    R = _ceil(cnt, PB).max(axis=0)            # [NBLK, 2] tiles, shared SPMD

    Tb = R.sum(axis=1)                        # tiles per block
    TT = int(Tb.sum())                        # total tiles per core
    tile_base = np.zeros(NBLK, np.int64)      # first global tile of block
    tile_base[1:] = np.cumsum(Tb)[:-1]

    # gather op layout (shared): ops are (block, half) chunks of <= TMAX tiles
    ops = []                                  # (blk, gt0, ntiles, half)
    for b in range(NBLK):
        t0 = int(tile_base[b])
        for hf in range(2):
            r = int(R[b, hf])
            while r > 0:
                nt = min(r, TMAX)
                ops.append((b, t0, nt, hf))
                t0 += nt
                r -= nt
    idx_cols = sum(nt * PB // 16 for (_, _, nt, _) in ops)

    starts = np.zeros(NC * NBLK * 2 + 1, np.int64)
    starts[1:] = np.cumsum(cnt.ravel())

    per_core = []
    for c in range(NC):
        gidx = np.zeros((16, idx_cols), np.int16)
        S = np.zeros((PB, TT * PB), bf16)
        ST = np.zeros((PB, TT * PB), bf16)

        col = 0
        for (b, gt0, nt, hf) in ops:
            kk = (c * NBLK + b) * 2 + hf
            seg = order[starts[kk]:starts[kk + 1]]
            # slots covered by this op: global tiles gt0..gt0+nt
            base_slot = (gt0 - tile_base[b] - (R[b, 0] if hf else 0)) * PB
            lo_e = base_slot
            hi_e = min(base_slot + nt * PB, len(seg))
            rows = np.zeros(nt * PB, np.int64)   # table rows (junk -> 0)
            if hi_e > lo_e:
                ee = seg[lo_e:hi_e]
                rows[:hi_e - lo_e] = src[ee] - (SPLIT if hf else 0)
                # slot i in op -> tile gt0 + i//128, partition i%128
                i = np.arange(hi_e - lo_e)
                p_slot = i % PB
                t_slot = gt0 + i // PB
                S[p_slot, t_slot * PB + dib[ee]] = 1.0
                ST[dib[ee], t_slot * PB + p_slot] = 1.0
            ncol = nt * PB // 16
            gidx[:, col:col + ncol] = rows.reshape(ncol, 16).T
            col += ncol
        gidx = np.tile(gidx, (8, 1))

        # graph selector [128, NBLK*GSH] f32 and graph node ranges
        gsel = np.zeros((PB, NBLK * GSH), np.float32)
        nloc = np.arange(NSH)
        gloc = gid[c * NSH:(c + 1) * NSH] - c * GSH
        gsel[nloc % PB, (nloc // PB) * GSH + gloc] = 1.0
        granges = [(int(bounds[c * GSH + g] - c * NSH),
                    int((bounds[c * GSH + g + 1] if c * GSH + g + 1 < G else N)
                        - c * NSH))
                   for g in range(GSH)]

        hT = np.zeros((PB, XR), bf16)
        hT[:, :NSH] = h[c * NSH:(c + 1) * NSH].T.astype(bf16)

        per_core.append(dict(gidx=gidx, S=S, ST=ST, gsel=gsel, hT=hT,
                             granges=granges))

    # ---- weights
    def aug(W, al, ar):
        W = np.asarray(W, np.float32)
        al = np.asarray(al, np.float32)
        ar = np.asarray(ar, np.float32)
        A = np.zeros((KH, 2 * K), np.float32)
        for k in range(K):
            A[k * H:(k + 1) * H, k] = al[k]
            A[k * H:(k + 1) * H, K + k] = ar[k]
        return np.concatenate([W, W @ A], axis=1).astype(bf16)  # [F, KH+2K]

    wd = dict(
        W0a=aug(inputs["W0"], inputs["al0"], inputs["ar0"]),
        W1a=aug(inputs["W1"], inputs["al1"], inputs["ar1"]),
        W2a=aug(inputs["W2"], inputs["al2"], inputs["ar2"]),
        resW0=np.asarray(inputs["resW0"], np.float32).astype(bf16),
        Wg_rep=np.tile(np.asarray(inputs["Wg"], np.float32).reshape(1, H),
                       (PB, 1)),
        Wm1=np.asarray(inputs["Wm1"], np.float32),            # [2H, H]
        Wm2=np.asarray(inputs["Wm2"], np.float32),            # [H, 1]
        bm1=np.asarray(inputs["bm1"], np.float32).reshape(H, 1),
        ident=np.eye(PB, dtype=np.float32),
    )
    bn_a = (np.asarray(inputs["bn_g"], np.float32)
            / np.sqrt(np.asarray(inputs["bn_v"], np.float32) + EPS))
    wd["bn_a"] = bn_a.reshape(H, 1)
    wd["bn_b"] = (np.asarray(inputs["bn_b"], np.float32)
                  - np.asarray(inputs["bn_m"], np.float32) * bn_a).reshape(H, 1)
    biases = [np.asarray(inputs[k], np.float32) for k in ("b0", "b1", "b2")]
    use_bias = [bool(np.any(b != 0)) for b in biases]
    for li in range(3):
        if use_bias[li]:
            wd[f"brep{li}"] = np.tile(biases[li].reshape(1, KH), (PB, 1))

    meta = dict(N=N, F=F, E=E, G=G, K=K, H=H, KH=KH, NSH=NSH, GSH=GSH,
                NBLK=NBLK, XR=XR, WROW=WROW, TT=TT, R=R, Tb=Tb,
                tile_base=tile_base, ops=ops, idx_cols=idx_cols,
                use_bias=use_bias,
                bg=float(np.asarray(inputs["bg"]).ravel()[0]),
                bm2=float(np.asarray(inputs["bm2"]).ravel()[0]),
                granges=per_core[0]["granges"])

    in_maps = []
    for c in range(NC):
        m = dict(per_core[c])
        gr = m.pop("granges")
        m.update(wd)
        in_maps.append(m)
    for pc in per_core[1:]:
        assert pc["granges"] == per_core[0]["granges"]
    return in_maps, meta


# --------------------------------------------------------------------------
# device program
# --------------------------------------------------------------------------

def _build(meta):
    from concourse import bass, bacc, tile, mybir

    f32 = mybir.dt.float32
    bft = mybir.dt.bfloat16
    i16 = mybir.dt.int16
    AF = mybir.ActivationFunctionType
    OP = mybir.AluOpType

    N, F, KH, K, H = meta["N"], meta["F"], meta["KH"], meta["K"], meta["H"]
    NSH, GSH, NBLK, XR = meta["NSH"], meta["GSH"], meta["NBLK"], meta["XR"]
    WROW, TT = meta["WROW"], meta["TT"]
    R, Tb, tile_base, ops = meta["R"], meta["Tb"], meta["tile_base"], meta["ops"]
    use_bias = meta["use_bias"]
    granges = meta["granges"]
    NW = KH + 2 * K                      # 264 dense out cols

    nc = bacc.Bacc("TRN2", target_bir_lowering=False, debug=False,
                   enable_asserts=False, num_devices=NC, num_swdge_queues=NQ)

    # ---- I/O
    ein = {}
    for name, shape, dt in [
        ("gidx", [PB, meta["idx_cols"]], i16),
        ("S", [PB, TT * PB], bft),
        ("ST", [PB, TT * PB], bft),
        ("gsel", [PB, NBLK * GSH], f32),
        ("hT", [PB, XR], bft),
        ("W0a", [F, NW], bft),
        ("W1a", [KH, NW], bft),
        ("W2a", [KH, NW], bft),
        ("resW0", [F, KH], bft),
        ("Wg_rep", [PB, H], f32),
        ("Wm1", [2 * H, H], f32),
        ("Wm2", [H, 1], f32),
        ("bm1", [H, 1], f32),
        ("bn_a", [H, 1], f32),
        ("bn_b", [H, 1], f32),
        ("ident", [PB, PB], f32),
    ]:
        ein[name] = nc.dram_tensor(name, shape, dt, kind="ExternalInput")
    for li in range(3):
        if use_bias[li]:
            ein[f"brep{li}"] = nc.dram_tensor(f"brep{li}", [PB, KH], f32,
                                              kind="ExternalInput")
    out_ext = nc.dram_tensor("out", [GSH, 1], f32, kind="ExternalOutput")
    dbg = {}
    import os as _os
    DBG = bool(int(_os.environ.get("KGAT_DEBUG", "0")))
    if DBG:
        for li in range(3):
            dbg[f"x{li+1}"] = nc.dram_tensor(
                f"dbg_x{li+1}", [PB, NBLK * (KH if li < 2 else H)], f32,
                kind="ExternalOutput")
        dbg["tab"] = nc.dram_tensor("dbg_tab", [N, WROW], bft,
                                    kind="ExternalOutput")
        dbg["M"] = nc.dram_tensor("dbg_M", [PB, 16 * (KH + K)], f32,
                                  kind="ExternalOutput")
        dbg["ere"] = nc.dram_tensor("dbg_ere", [PB, 16 * K], f32,
                                    kind="ExternalOutput")

    # ---- internal DRAM
    cc_in = [nc.dram_tensor(f"cc_in{li}", [NSH, WROW], bft, kind="Internal")
             for li in range(3)]
    tables = [nc.dram_tensor(f"table{li}", [N, WROW], bft, kind="Internal",
                             addr_space="Shared") for li in range(3)]
    x_hbm = [nc.dram_tensor(f"x_hbm{li}", [XR, KH], bft, kind="Internal")
             for li in range(2)]

    qctr = [0]

    def next_q():
        q = qctr[0] % NQ
        qctr[0] += 1
        return q

    with tile.TileContext(nc) as tc:
        with (
            tc.tile_pool(name="const", bufs=1) as cp,
            tc.tile_pool(name="state", bufs=1) as st,
            tc.tile_pool(name="sb", bufs=3) as sb,
            tc.tile_pool(name="gbuf", bufs=2) as gb,
            tc.tile_pool(name="xp", bufs=1) as xp,
            tc.tile_pool(name="ps", bufs=2, space="PSUM") as ps,
            tc.tile_pool(name="ps2", bufs=2, space="PSUM") as ps2,
        ):
            # ---- constants to SBUF
            c_t = {}
            for name in ["gsel", "hT", "resW0", "Wg_rep",
                         "Wm1", "Wm2", "bm1", "bn_a", "bn_b", "ident"]:
                tsr = ein[name]
                t = cp.tile(list(tsr.shape), tsr.dtype, tag=f"c_{name}")
                nc.sync.dma_start(out=t[:], in_=tsr[:])
                c_t[name] = t
            W_chunks = {}
            for name in ["W0a", "W1a", "W2a"]:
                tsr = ein[name]
                nch = tsr.shape[0] // PB
                lst = []
                for kc in range(nch):
                    t = cp.tile([PB, NW], bft, tag=f"c_{name}_{kc}")
                    nc.sync.dma_start(out=t[:],
                                      in_=tsr[kc * PB:(kc + 1) * PB, :])
                    lst.append(t)
                W_chunks[name] = lst
            brep = {}
            for li in range(3):
                if use_bias[li]:
                    t = cp.tile([PB, KH], f32, tag=f"c_brep{li}")
                    nc.sync.dma_start(out=t[:], in_=ein[f"brep{li}"][:])
                    brep[li] = t
            gidx_sb = cp.tile([PB, meta["idx_cols"]], i16, tag="c_gidx")
            nc.sync.dma_start(out=gidx_sb[:], in_=ein["gidx"][:])

            # ---- persistent state
            x_sh = st.tile([PB, NBLK, KH], f32, tag="x_sh")
            er_sh = st.tile([PB, NBLK, 2 * K], bft, tag="er_sh")
            x3_sb = st.tile([PB, NBLK, H], f32, tag="x3")
            x3T = st.tile([H, XR], f32, tag="x3T")
            zero128 = cp.tile([PB, KH], bft, tag="zeros")
            nc.gpsimd.memset(zero128[:], 0.0)
            # zero the x_hbm pad rows once
            for li in range(2):
                if XR > NSH:
                    nc.sync.dma_start(out=x_hbm[li][NSH:XR, :],
                                      in_=zero128[:XR - NSH, :])

            op_col = {}
            col = 0
            for oi, (b, gt0, nt, hf) in enumerate(ops):
                op_col[oi] = col
                col += nt * PB // 16

            # ================= layers =================
            for li in range(3):
                Wa = W_chunks[["W0a", "W1a", "W2a"][li]]
                tab = tables[li]
                cci = cc_in[li]

                # ---- dense: z shard -> cc_in
                if li == 0:
                    xT_chunks = [c_t["hT"]]
                else:
                    xT_chunks = []
                    for kc in range(2):
                        t = xp.tile([PB, XR], bft, tag=f"xT{kc}")
                        nc.sync.dma_start(
                            out=t[:], in_=x_hbm[li - 1][:, kc * PB:(kc + 1) * PB],
                            transpose=True)
                        xT_chunks.append(t)
                for nb in range(NBLK):
                    r1 = min((nb + 1) * PB, NSH) - nb * PB
                    zps = ps.tile([PB, NW], f32, tag="zps")
                    for kc, xT in enumerate(xT_chunks):
                        nc.tensor.matmul(
                            zps[:], lhsT=xT[:, nb * PB:nb * PB + PB],
                            rhs=Wa[kc][:],
                            start=(kc == 0), stop=(kc == len(xT_chunks) - 1))
                    zsb = sb.tile([PB, NW], bft, tag="zsb")
                    nc.vector.tensor_copy(out=zsb[:], in_=zps[:])
                    nc.vector.tensor_copy(out=er_sh[:, nb, :],
                                          in_=zps[:, KH:KH + 2 * K])
                    nc.sync.dma_start(out=cci[nb * PB:nb * PB + r1, :NW],
                                      in_=zsb[:r1, :])
                # ---- exchange
                nc.gpsimd.collective_compute(
                    "AllGather", OP.bypass,
                    replica_groups=[list(range(NC))],
                    ins=[cci[:]], outs=[tab[:]],
                )

                if DBG and li == 0:
                    nc.sync.dma_start(out=dbg["tab"][:], in_=tab[:])
                # ---- edge phase
                for nb in range(NBLK):
                    T = int(Tb[nb])
                    if T == 0:
                        continue
                    t0 = int(tile_base[nb])
                    G_t = gb.tile([PB, T, WROW], bft, tag="G")
                    # gathers (ops of this block)
                    for oi, (b, gt0, nt, hf) in enumerate(ops):
                        if b != nb:
                            continue
                        lt = gt0 - t0
                        src_ap = tab[SPLIT:, :] if hf else tab[:, :]
                        nc.gpsimd.dma_gather(
                            G_t[:, lt:lt + nt, :], src_ap,
                            gidx_sb[:, op_col[oi]:op_col[oi] + nt * PB // 16],
                            num_idxs=nt * PB, num_idxs_reg=nt * PB,
                            elem_size=WROW, queue_num=next_q(),
                        )
                    S_t = gb.tile([PB, T * PB], bft, tag="S")
                    ST_t = gb.tile([PB, T * PB], bft, tag="ST")
                    nc.sync.dma_start(
                        out=S_t[:], in_=ein["S"][:, t0 * PB:(t0 + T) * PB])
                    nc.sync.dma_start(
                        out=ST_t[:], in_=ein["ST"][:, t0 * PB:(t0 + T) * PB])

                    # er broadcast to edges: erps[:, t, :] = ST_t.T @ er_blk
                    erps = ps2.tile([PB, T, K], f32, tag="erps")
                    for t in range(T):
                        nc.tensor.matmul(
                            erps[:, t, :], lhsT=ST_t[:, t * PB:(t + 1) * PB],
                            rhs=er_sh[:, nb, K:2 * K], start=True, stop=True)
                    er_e = sb.tile([PB, T, K], bft, tag="er_e")
                    nc.scalar.activation(out=er_e[:], in_=erps[:], func=AF.Copy)
                    # e_pre = el[src] + er[dst]
                    epre = sb.tile([PB, T, K], bft, tag="epre")
                    nc.vector.tensor_tensor(
                        out=epre[:], in0=G_t[:, :, KH:KH + K], in1=er_e[:],
                        op=OP.add)
                    # w = exp(leaky_relu(e_pre))
                    lr2 = sb.tile([PB, T, K], bft, tag="lr2")
                    nc.vector.tensor_scalar(out=lr2[:], in0=epre[:],
                                            scalar1=0.2, scalar2=None,
                                            op0=OP.mult)
                    lr = sb.tile([PB, T, K], bft, tag="lr")
                    nc.vector.tensor_tensor(out=lr[:], in0=epre[:], in1=lr2[:],
                                            op=OP.max)
                    M_t = gb.tile([PB, T, KH + K], bft, tag="M")
                    nc.scalar.activation(out=M_t[:, :, KH:KH + K], in_=lr[:],
                                         func=AF.Exp)
                    # messages = w (bcast per head) * z
                    nc.vector.tensor_tensor(
                        out=M_t[:, :, :KH].rearrange("p t (k h) -> p t k h", k=K),
                        in0=G_t[:, :, :KH].rearrange("p t (k h) -> p t k h", k=K),
                        in1=M_t[:, :, KH:KH + K].unsqueeze(-1)
                            .to_broadcast([PB, T, K, H]),
                        op=OP.mult)
                    if DBG and li == 0 and nb == 0:
                        mt = min(T, 16)
                        mdump = sb.tile([PB, 16 * (KH + K)], f32, tag="mdump")
                        nc.gpsimd.memset(mdump[:], 0.0)
                        nc.vector.tensor_copy(
                            out=mdump[:, :mt * (KH + K)],
                            in_=M_t[:, :mt, :].rearrange("p t w -> p (t w)"))
                        nc.sync.dma_start(out=dbg["M"][:], in_=mdump[:])
                        edump = sb.tile([PB, 16 * K], f32, tag="edump")
                        nc.gpsimd.memset(edump[:], 0.0)
                        nc.vector.tensor_copy(
                            out=edump[:, :mt * K],
                            in_=er_e[:].rearrange("p t k -> p (t k)"))
                        nc.sync.dma_start(out=dbg["ere"][:], in_=edump[:])
                    # aggregate
                    U = ps.tile([PB, KH + K], f32, tag="U")
                    for t in range(T):
                        nc.tensor.matmul(U[:], lhsT=S_t[:, t * PB:(t + 1) * PB],
                                         rhs=M_t[:, t, :],
                                         start=(t == 0), stop=(t == T - 1))
                    # epilogue
                    seps = sb.tile([PB, K], f32, tag="seps")
                    nc.scalar.activation(out=seps[:], in_=U[:, KH:KH + K],
                                         func=AF.Copy, bias=1e-30)
                    invs = sb.tile([PB, K], f32, tag="invs")
                    nc.vector.reciprocal(out=invs[:], in_=seps[:])
                    xo = sb.tile([PB, KH], f32, tag="xo")
                    nc.vector.tensor_tensor(
                        out=xo[:].rearrange("p (k h) -> p k h", k=K),
                        in0=U[:, :KH].rearrange("p (k h) -> p k h", k=K),
                        in1=invs[:].unsqueeze(-1).to_broadcast([PB, K, H]),
                        op=OP.mult)
                    if li == 0:
                        res = ps2.tile([PB, KH], f32, tag="res")
                        nc.tensor.matmul(res[:],
                                         lhsT=c_t["hT"][:, nb * PB:nb * PB + PB],
                                         rhs=c_t["resW0"][:], start=True,
                                         stop=True)
                        nc.vector.tensor_tensor(out=xo[:], in0=xo[:],
                                                in1=res[:], op=OP.add)
                    else:
                        nc.vector.tensor_tensor(out=xo[:], in0=xo[:],
                                                in1=x_sh[:, nb, :], op=OP.add)
                    if use_bias[li]:
                        nc.vector.tensor_tensor(out=xo[:], in0=xo[:],
                                                in1=brep[li][:], op=OP.add)
                    if li < 2:
                        nc.scalar.activation(out=x_sh[:, nb, :], in_=xo[:],
                                             func=AF.Relu)
                        r1 = min((nb + 1) * PB, NSH) - nb * PB
                        nc.gpsimd.dma_start(
                            out=x_hbm[li][nb * PB:nb * PB + r1, :],
                            in_=x_sh[:r1, nb, :])
                    else:
                        xr = sb.tile([PB, KH], f32, tag="xr")
                        nc.scalar.activation(out=xr[:], in_=xo[:], func=AF.Relu)
                        # mean over heads
                        m1 = sb.tile([PB, H], f32, tag="m1")
                        nc.vector.tensor_tensor(out=m1[:], in0=xr[:, 0:H],
                                                in1=xr[:, H:2 * H], op=OP.add)
                        m2 = sb.tile([PB, H], f32, tag="m2")
                        nc.vector.tensor_tensor(out=m2[:], in0=xr[:, 2 * H:3 * H],
                                                in1=xr[:, 3 * H:4 * H], op=OP.add)
                        nc.vector.tensor_tensor(out=m1[:], in0=m1[:], in1=m2[:],
                                                op=OP.add)
                        nc.scalar.activation(out=x3_sb[:, nb, :], in_=m1[:],
                                             func=AF.Copy, scale=0.25)

                if DBG:
                    if li < 2:
                        nc.sync.dma_start(
                            out=dbg[f"x{li+1}"][:],
                            in_=x_sh[:].rearrange("p b w -> p (b w)"))
                    else:
                        nc.sync.dma_start(
                            out=dbg["x3"][:],
                            in_=x3_sb[:].rearrange("p b w -> p (b w)"))

            # ================= readout =================
            gsps = ps.tile([GSH, H], f32, tag="zps")
            for nb in range(NBLK):
                # w = sigmoid(x3 @ Wg + bg) via row-dot
                wx = sb.tile([PB, H], f32, tag="wx")
                nc.vector.tensor_tensor(out=wx[:], in0=x3_sb[:, nb, :],
                                        in1=c_t["Wg_rep"][:], op=OP.mult)
                wcol = sb.tile([PB, 1], f32, tag="wcol")
                nc.vector.reduce_sum(out=wcol[:], in_=wx[:],
                                     axis=mybir.AxisListType.X)
                wsig = sb.tile([PB, 1], f32, tag="wsig")
                nc.scalar.activation(out=wsig[:], in_=wcol[:], func=AF.Sigmoid,
                                     bias=meta["bg"])
                wx3 = sb.tile([PB, H], f32, tag="wx3")
                nc.vector.tensor_tensor(
                    out=wx3[:], in0=x3_sb[:, nb, :],
                    in1=wsig[:].to_broadcast([PB, H]), op=OP.mult)
                nc.tensor.matmul(gsps[:], lhsT=c_t["gsel"][:, nb * GSH:(nb + 1) * GSH],
                                 rhs=wx3[:], start=(nb == 0),
                                 stop=(nb == NBLK - 1))
                # x3 transpose for segment max
                tps = ps2.tile([H, PB], f32, tag="res")
                nc.tensor.transpose(tps[:], x3_sb[:, nb, :], c_t["ident"][:])
                nc.scalar.activation(out=x3T[:, nb * PB:(nb + 1) * PB],
                                     in_=tps[:], func=AF.Copy)
            # segment max per graph
            gmT = st.tile([H, GSH], f32, tag="gmT")
            for g, (s, e) in enumerate(granges):
                nc.vector.reduce_max(out=gmT[:, g:g + 1], in_=x3T[:, s:e],
                                     axis=mybir.AxisListType.X)
            # gfT = [gs^T ; gmT]  [2H, GSH]
            gs_sb = sb.tile([GSH, H], f32, tag="gs_sb")
            nc.vector.tensor_copy(out=gs_sb[:], in_=gsps[:])
            gsT = ps2.tile([H, GSH], f32, tag="erps")
            nc.tensor.transpose(gsT[:], gs_sb[:], c_t["ident"][:GSH, :GSH])
            gfT = sb.tile([2 * H, GSH], f32, tag="gfT")
            nc.scalar.activation(out=gfT[:H, :], in_=gsT[:], func=AF.Copy)
            nc.vector.tensor_copy(out=gfT[H:, :], in_=gmT[:])
            # y1T = relu(Wm1^T @ gfT + bm1); BN; out = sigmoid(y2T^T @ Wm2 + bm2)
            y1ps = ps.tile([H, GSH], f32, tag="zps")
            nc.tensor.matmul(y1ps[:], lhsT=c_t["Wm1"][:], rhs=gfT[:],
                             start=True, stop=True)
            y1 = sb.tile([H, GSH], f32, tag="y1")
            nc.scalar.activation(out=y1[:], in_=y1ps[:], func=AF.Relu,
                                 bias=c_t["bm1"][:])
            y2 = sb.tile([H, GSH], f32, tag="y2")
            nc.scalar.activation(out=y2[:], in_=y1[:], func=AF.Identity,
                                 scale=c_t["bn_a"][:], bias=c_t["bn_b"][:])
            ops_ = ps.tile([GSH, 1], f32, tag="U")
            nc.tensor.matmul(ops_[:], lhsT=y2[:], rhs=c_t["Wm2"][:],
                             start=True, stop=True)
            osb = sb.tile([GSH, 1], f32, tag="osb")
            nc.scalar.activation(out=osb[:], in_=ops_[:], func=AF.Sigmoid,
                                 bias=meta["bm2"])
            nc.sync.dma_start(out=out_ext[:], in_=osb[:])

    nc.compile()
    return nc


# --------------------------------------------------------------------------

def kernel(**inputs):
    from concourse.bass_utils import run_bass_kernel_spmd

    in_maps, meta = _prep(inputs)
    key = meta["N"], meta["E"], meta["G"], meta["TT"]
    if key not in _CACHE:
        _CACHE[key] = _build(meta)
    nc = _CACHE[key]
    res = run_bass_kernel_spmd(nc, in_maps, core_ids=list(range(NC)))
    out = np.concatenate([res.results[c]["out"] for c in range(NC)], axis=0)
    return out.astype(np.float32)


# revision 11
# speedup vs baseline: 1.0290x; 1.0290x over previous
"""EnhancedGAT (3-layer GAT + WeightedSumAndMax readout + MLP) on 8 TRN2
NeuronCores.

Sharding: nodes/graphs partitioned 8 ways (graph-aligned); GAT/MLP weights
replicated. Per layer: node-sharded dense projection -> AllGather of a compact
bf16 node table [z|el|er] -> edge phase with dma_gather row gathers (by src)
and selection-matrix PSUM aggregation (by dst). Selection matrices S / S_T are
host-precomputed constants (edge structure is static per call).

Self-contained: hardcodes the problem geometry; host side only reorders /
packs indices and casts weights (no FLOPs of the model are done on host).
"""
import math
import numpy as np
import ml_dtypes

bf16 = ml_dtypes.bfloat16

NC = 8           # cores
PB = 128         # partition/block size
TMAX = 8         # max tiles per dma_gather op (num_idxs <= 1024)
SPLIT = 32768    # int16 index range split for the gather table
NQ = 4           # SWDGE queues
EPS = 1e-5

_CACHE = {}


# --------------------------------------------------------------------------
# host-side preprocessing
# --------------------------------------------------------------------------

def _ceil(a, b):
    return -(-a // b)


def _pack_idx16(lst):
    """gather list -> [128, n/16] int16 tile (16-wrapped, replicated x8)."""
    n = len(lst)
    assert n % 16 == 0
    a = np.asarray(lst, np.int16).reshape(n // 16, 16).T.copy()  # [16, n/16]
    return np.tile(a, (8, 1))


def _prep(inputs):
    h = np.asarray(inputs["h"], np.float32)
    src = np.asarray(inputs["src"]).astype(np.int64).ravel()
    dst = np.asarray(inputs["dst"]).astype(np.int64).ravel()
    gid = np.asarray(inputs["graph_ids"]).astype(np.int64).ravel()

    N, F = h.shape
    E = src.shape[0]
    G = int(gid.max()) + 1
    K, H = np.asarray(inputs["al0"]).shape
    KH = K * H

    assert N % NC == 0 and G % NC == 0 and F == PB and KH == 2 * PB
    NSH = N // NC
    GSH = G // NC
    NBLK = _ceil(NSH, PB)
    XR = NBLK * PB           # padded x rows per shard
    WROW = 3 * KH // 2       # 384 bf16 = 768B table row: [z 256|el 4|er 4|pad]
    assert (WROW * 2) % 256 == 0

    # graph boundaries must align with node shards
    bounds = np.searchsorted(gid, np.arange(G))
    assert all(bounds[c * GSH] == c * NSH for c in range(NC))

    # ---- edge partition: core = dst // NSH, block = local dst // 128
    core_of = dst // NSH
    dl = dst - core_of * NSH
    blk = dl // PB
    dib = (dl % PB).astype(np.int32)          # dst-in-block column
    half = (src >= SPLIT).astype(np.int64)
    key = (core_of * NBLK + blk) * 2 + half
    order = np.argsort(key, kind="stable")
    cnt = np.bincount(key, minlength=NC * NBLK * 2).reshape(NC, NBLK, 2)
    R = _ceil(cnt, PB).max(axis=0)            # [NBLK, 2] tiles, shared SPMD

    Tb = R.sum(axis=1)                        # tiles per block
    TT = int(Tb.sum())                        # total tiles per core
    tile_base = np.zeros(NBLK, np.int64)      # first global tile of block
    tile_base[1:] = np.cumsum(Tb)[:-1]

    # gather op layout (shared): ops are (block, half) chunks of <= TMAX tiles
    ops = []                                  # (blk, gt0, ntiles, half)
    for b in range(NBLK):
        t0 = int(tile_base[b])
        for hf in range(2):
            r = int(R[b, hf])
            while r > 0:
                nt = min(r, TMAX)
                ops.append((b, t0, nt, hf))
                t0 += nt
                r -= nt
    idx_cols = sum(nt * PB // 16 for (_, _, nt, _) in ops)

    starts = np.zeros(NC * NBLK * 2 + 1, np.int64)
    starts[1:] = np.cumsum(cnt.ravel())

    per_core = []
    for c in range(NC):
        gidx = np.zeros((16, idx_cols), np.int16)
        S = np.zeros((PB, TT * PB), bf16)
        ST = np.zeros((PB, TT * PB), bf16)

        col = 0
        for (b, gt0, nt, hf) in ops:
            kk = (c * NBLK + b) * 2 + hf
            seg = order[starts[kk]:starts[kk + 1]]
            # slots covered by this op: global tiles gt0..gt0+nt
            base_slot = (gt0 - tile_base[b] - (R[b, 0] if hf else 0)) * PB
            lo_e = base_slot
            hi_e = min(base_slot + nt * PB, len(seg))
            rows = np.zeros(nt * PB, np.int64)   # table rows (junk -> 0)
            if hi_e > lo_e:
                ee = seg[lo_e:hi_e]
                rows[:hi_e - lo_e] = src[ee] - (SPLIT if hf else 0)
                # slot i in op -> tile gt0 + i//128, partition i%128
                i = np.arange(hi_e - lo_e)
                p_slot = i % PB
                t_slot = gt0 + i // PB
                S[p_slot, t_slot * PB + dib[ee]] = 1.0
                ST[dib[ee], t_slot * PB + p_slot] = 1.0
            ncol = nt * PB // 16
            gidx[:, col:col + ncol] = rows.reshape(ncol, 16).T
            col += ncol
        gidx = np.tile(gidx, (8, 1))

        # graph selector [128, NBLK*GSH] f32 and graph node ranges
        gsel = np.zeros((PB, NBLK * GSH), bf16)
        nloc = np.arange(NSH)
        gloc = gid[c * NSH:(c + 1) * NSH] - c * GSH
        gsel[nloc % PB, (nloc // PB) * GSH + gloc] = 1.0
        granges = [(int(bounds[c * GSH + g] - c * NSH),
                    int((bounds[c * GSH + g + 1] if c * GSH + g + 1 < G else N)
                        - c * NSH))
                   for g in range(GSH)]

        hT = np.zeros((PB, XR), bf16)
        hT[:, :NSH] = h[c * NSH:(c + 1) * NSH].T.astype(bf16)

        per_core.append(dict(gidx=gidx, S=S, ST=ST, gsel=gsel, hT=hT,
                             granges=granges))

    # ---- weights
    def aug(W, al, ar):
        W = np.asarray(W, np.float32)
        al = np.asarray(al, np.float32)
        ar = np.asarray(ar, np.float32)
        A = np.zeros((KH, 2 * K), np.float32)
        for k in range(K):
            A[k * H:(k + 1) * H, k] = al[k]
            A[k * H:(k + 1) * H, K + k] = ar[k]
        return np.concatenate([W, W @ A], axis=1).astype(bf16)  # [F, KH+2K]

    wd = dict(
        W0a=aug(inputs["W0"], inputs["al0"], inputs["ar0"]),
        W1a=aug(inputs["W1"], inputs["al1"], inputs["ar1"]),
        W2a=aug(inputs["W2"], inputs["al2"], inputs["ar2"]),
        resW0=np.asarray(inputs["resW0"], np.float32).astype(bf16),
        Wg_rep=np.tile(np.asarray(inputs["Wg"], np.float32).reshape(1, H),
                       (PB, 1)),
        Wm1=np.asarray(inputs["Wm1"], np.float32),            # [2H, H]
        Wm2=np.asarray(inputs["Wm2"], np.float32),            # [H, 1]
        bm1=np.asarray(inputs["bm1"], np.float32).reshape(H, 1),
        ident=np.eye(PB, dtype=np.float32),
    )
    bn_a = (np.asarray(inputs["bn_g"], np.float32)
            / np.sqrt(np.asarray(inputs["bn_v"], np.float32) + EPS))
    wd["bn_a"] = bn_a.reshape(H, 1)
    wd["bn_b"] = (np.asarray(inputs["bn_b"], np.float32)
                  - np.asarray(inputs["bn_m"], np.float32) * bn_a).reshape(H, 1)
    biases = [np.asarray(inputs[k], np.float32) for k in ("b0", "b1", "b2")]
    use_bias = [bool(np.any(b != 0)) for b in biases]
    for li in range(3):
        if use_bias[li]:
            wd[f"brep{li}"] = np.tile(biases[li].reshape(1, KH), (PB, 1))

    meta = dict(N=N, F=F, E=E, G=G, K=K, H=H, KH=KH, NSH=NSH, GSH=GSH,
                NBLK=NBLK, XR=XR, WROW=WROW, TT=TT, R=R, Tb=Tb,
                tile_base=tile_base, ops=ops, idx_cols=idx_cols,
                use_bias=use_bias,
                bg=float(np.asarray(inputs["bg"]).ravel()[0]),
                bm2=float(np.asarray(inputs["bm2"]).ravel()[0]),
                granges=per_core[0]["granges"])

    in_maps = []
    for c in range(NC):
        m = dict(per_core[c])
        gr = m.pop("granges")
        m.update(wd)
        in_maps.append(m)
    for pc in per_core[1:]:
        assert pc["granges"] == per_core[0]["granges"]
    return in_maps, meta


# --------------------------------------------------------------------------
# device program
# --------------------------------------------------------------------------

def _build(meta):
    from concourse import bass, bacc, tile, mybir

    f32 = mybir.dt.float32
    bft = mybir.dt.bfloat16
    i16 = mybir.dt.int16
    AF = mybir.ActivationFunctionType
    OP = mybir.AluOpType

    N, F, KH, K, H = meta["N"], meta["F"], meta["KH"], meta["K"], meta["H"]
    NSH, GSH, NBLK, XR = meta["NSH"], meta["GSH"], meta["NBLK"], meta["XR"]
    WROW, TT = meta["WROW"], meta["TT"]
    R, Tb, tile_base, ops = meta["R"], meta["Tb"], meta["tile_base"], meta["ops"]
    use_bias = meta["use_bias"]
    granges = meta["granges"]
    NW = KH + 2 * K                      # 264 dense out cols

    nc = bacc.Bacc("TRN2", target_bir_lowering=False, debug=False,
                   enable_asserts=False, num_devices=NC, num_swdge_queues=NQ)

    # ---- I/O
    ein = {}
    for name, shape, dt in [
        ("gidx", [PB, meta["idx_cols"]], i16),
        ("S", [PB, TT * PB], bft),
        ("ST", [PB, TT * PB], bft),
        ("gsel", [PB, NBLK * GSH], bft),
        ("hT", [PB, XR], bft),
        ("W0a", [F, NW], bft),
        ("W1a", [KH, NW], bft),
        ("W2a", [KH, NW], bft),
        ("resW0", [F, KH], bft),
        ("Wg_rep", [PB, H], f32),
        ("Wm1", [2 * H, H], f32),
        ("Wm2", [H, 1], f32),
        ("bm1", [H, 1], f32),
        ("bn_a", [H, 1], f32),
        ("bn_b", [H, 1], f32),
        ("ident", [PB, PB], f32),
    ]:
        ein[name] = nc.dram_tensor(name, shape, dt, kind="ExternalInput")
    for li in range(3):
        if use_bias[li]:
            ein[f"brep{li}"] = nc.dram_tensor(f"brep{li}", [PB, KH], f32,
                                              kind="ExternalInput")
    out_ext = nc.dram_tensor("out", [GSH, 1], f32, kind="ExternalOutput")
    dbg = {}
    import os as _os
    DBG = bool(int(_os.environ.get("KGAT_DEBUG", "0")))
    if DBG:
        for li in range(3):
            dbg[f"x{li+1}"] = nc.dram_tensor(
                f"dbg_x{li+1}", [PB, NBLK * (KH if li < 2 else H)], f32,
                kind="ExternalOutput")
        dbg["tab"] = nc.dram_tensor("dbg_tab", [N, WROW], bft,
                                    kind="ExternalOutput")
        dbg["M"] = nc.dram_tensor("dbg_M", [PB, 16 * (KH + K)], f32,
                                  kind="ExternalOutput")
        dbg["ere"] = nc.dram_tensor("dbg_ere", [PB, 16 * K], f32,
                                    kind="ExternalOutput")

    # ---- internal DRAM
    cc_in = [nc.dram_tensor(f"cc_in{li}", [NSH, WROW], bft, kind="Internal")
             for li in range(3)]
    tables = [nc.dram_tensor(f"table{li}", [N, WROW], bft, kind="Internal",
                             addr_space="Shared") for li in range(3)]
    x_hbm = [nc.dram_tensor(f"x_hbm{li}", [XR, KH], bft, kind="Internal")
             for li in range(2)]

    qctr = [0]

    def next_q():
        q = qctr[0] % NQ
        qctr[0] += 1
        return q

    with tile.TileContext(nc) as tc:
        with (
            tc.tile_pool(name="const", bufs=1) as cp,
            tc.tile_pool(name="state", bufs=1) as st,
            tc.tile_pool(name="sb", bufs=3) as sb,
            tc.tile_pool(name="gbuf", bufs=2) as gb,
            tc.tile_pool(name="xp", bufs=1) as xp,
            tc.tile_pool(name="ps", bufs=2, space="PSUM") as ps,
            tc.tile_pool(name="ps2", bufs=2, space="PSUM") as ps2,
        ):
            # ---- constants to SBUF
            c_t = {}
            for name in ["gsel", "hT", "resW0", "Wg_rep",
                         "Wm1", "Wm2", "bm1", "bn_a", "bn_b", "ident"]:
                tsr = ein[name]
                t = cp.tile(list(tsr.shape), tsr.dtype, tag=f"c_{name}")
                nc.sync.dma_start(out=t[:], in_=tsr[:])
                c_t[name] = t
            W_chunks = {}
            for name in ["W0a", "W1a", "W2a"]:
                tsr = ein[name]
                nch = tsr.shape[0] // PB
                lst = []
                for kc in range(nch):
                    t = cp.tile([PB, NW], bft, tag=f"c_{name}_{kc}")
                    nc.sync.dma_start(out=t[:],
                                      in_=tsr[kc * PB:(kc + 1) * PB, :])
                    lst.append(t)
                W_chunks[name] = lst
            brep = {}
            for li in range(3):
                if use_bias[li]:
                    t = cp.tile([PB, KH], f32, tag=f"c_brep{li}")
                    nc.sync.dma_start(out=t[:], in_=ein[f"brep{li}"][:])
                    brep[li] = t
            gidx_sb = cp.tile([PB, meta["idx_cols"]], i16, tag="c_gidx")
            nc.sync.dma_start(out=gidx_sb[:], in_=ein["gidx"][:])

            # ---- persistent state
            x_sh = st.tile([PB, NBLK, KH], bft, tag="x_sh")
            er_sh = st.tile([PB, NBLK, 2 * K], bft, tag="er_sh")
            x3_sb = st.tile([PB, NBLK, H], f32, tag="x3")
            x3T = st.tile([H, XR], f32, tag="x3T")
            zero128 = cp.tile([PB, KH], bft, tag="zeros")
            nc.gpsimd.memset(zero128[:], 0.0)
            # zero the x_hbm pad rows once
            for li in range(2):
                if XR > NSH:
                    nc.sync.dma_start(out=x_hbm[li][NSH:XR, :],
                                      in_=zero128[:XR - NSH, :])

            op_col = {}
            col = 0
            for oi, (b, gt0, nt, hf) in enumerate(ops):
                op_col[oi] = col
                col += nt * PB // 16

            # ================= layers =================
            for li in range(3):
                Wa = W_chunks[["W0a", "W1a", "W2a"][li]]
                tab = tables[li]
                cci = cc_in[li]

                # ---- dense: z shard -> cc_in
                sc_dense = nc.named_scope(f"L{li}_dense"); sc_dense.__enter__()
                if li == 0:
                    xT_chunks = [c_t["hT"]]
                else:
                    xT_chunks = []
                    for kc in range(2):
                        t = xp.tile([PB, XR], bft, tag=f"xT{kc}")
                        nc.sync.dma_start(
                            out=t[:], in_=x_hbm[li - 1][:, kc * PB:(kc + 1) * PB],
                            transpose=True)
                        xT_chunks.append(t)
                for nb in range(NBLK):
                    r1 = min((nb + 1) * PB, NSH) - nb * PB
                    zps = ps.tile([PB, NW], f32, tag="zps")
                    for kc, xT in enumerate(xT_chunks):
                        nc.tensor.matmul(
                            zps[:], lhsT=xT[:, nb * PB:nb * PB + PB],
                            rhs=Wa[kc][:],
                            start=(kc == 0), stop=(kc == len(xT_chunks) - 1))
                    zsb = sb.tile([PB, NW], bft, tag="zsb")
                    nc.vector.tensor_copy(out=zsb[:], in_=zps[:])
                    nc.vector.tensor_copy(out=er_sh[:, nb, :],
                                          in_=zps[:, KH:KH + 2 * K])
                    nc.sync.dma_start(out=cci[nb * PB:nb * PB + r1, :NW],
                                      in_=zsb[:r1, :])
                sc_dense.__exit__(None, None, None)
                # ---- exchange
                sc_ag = nc.named_scope(f"L{li}_ag"); sc_ag.__enter__()
                nc.gpsimd.collective_compute(
                    "AllGather", OP.bypass,
                    replica_groups=[list(range(NC))],
                    ins=[cci[:]], outs=[tab[:]],
                )

                if DBG and li == 0:
                    nc.sync.dma_start(out=dbg["tab"][:], in_=tab[:])
                sc_ag.__exit__(None, None, None)
                # ---- edge phase
                sc_edge = nc.named_scope(f"L{li}_edge"); sc_edge.__enter__()
                for nb in range(NBLK):
                    T = int(Tb[nb])
                    if T == 0:
                        continue
                    t0 = int(tile_base[nb])
                    G_t = gb.tile([PB, T, WROW], bft, tag="G")
                    # gathers (ops of this block)
                    for oi, (b, gt0, nt, hf) in enumerate(ops):
                        if b != nb:
                            continue
                        lt = gt0 - t0
                        src_ap = tab[SPLIT:, :] if hf else tab[:, :]
                        nc.gpsimd.dma_gather(
                            G_t[:, lt:lt + nt, :], src_ap,
                            gidx_sb[:, op_col[oi]:op_col[oi] + nt * PB // 16],
                            num_idxs=nt * PB, num_idxs_reg=nt * PB,
                            elem_size=WROW, queue_num=next_q(),
                        )
                    S_t = gb.tile([PB, T * PB], bft, tag="S")
                    ST_t = gb.tile([PB, T * PB], bft, tag="ST")
                    nc.sync.dma_start(
                        out=S_t[:], in_=ein["S"][:, t0 * PB:(t0 + T) * PB])
                    nc.sync.dma_start(
                        out=ST_t[:], in_=ein["ST"][:, t0 * PB:(t0 + T) * PB])

                    # er broadcast to edges: erps[:, t, :] = ST_t.T @ er_blk
                    erps = ps2.tile([PB, T, K], f32, tag="erps")
                    for t in range(T):
                        nc.tensor.matmul(
                            erps[:, t, :], lhsT=ST_t[:, t * PB:(t + 1) * PB],
                            rhs=er_sh[:, nb, K:2 * K], start=True, stop=True)
                    er_e = sb.tile([PB, T, K], bft, tag="er_e")
                    nc.scalar.activation(out=er_e[:], in_=erps[:], func=AF.Copy)
                    # e_pre = el[src] + er[dst]
                    epre = sb.tile([PB, T, K], bft, tag="epre")
                    nc.vector.tensor_tensor(
                        out=epre[:], in0=G_t[:, :, KH:KH + K], in1=er_e[:],
                        op=OP.add)
                    # w = exp(leaky_relu(e_pre))
                    lr2 = sb.tile([PB, T, K], bft, tag="lr2")
                    nc.vector.tensor_scalar(out=lr2[:], in0=epre[:],
                                            scalar1=0.2, scalar2=None,
                                            op0=OP.mult)
                    lr = sb.tile([PB, T, K], bft, tag="lr")
                    nc.vector.tensor_tensor(out=lr[:], in0=epre[:], in1=lr2[:],
                                            op=OP.max)
                    M_t = gb.tile([PB, T, KH + K], bft, tag="M")
                    nc.scalar.activation(out=M_t[:, :, KH:KH + K], in_=lr[:],
                                         func=AF.Exp)
                    # messages = w (bcast per head) * z
                    nc.vector.tensor_tensor(
                        out=M_t[:, :, :KH].rearrange("p t (k h) -> p t k h", k=K),
                        in0=G_t[:, :, :KH].rearrange("p t (k h) -> p t k h", k=K),
                        in1=M_t[:, :, KH:KH + K].unsqueeze(-1)
                            .to_broadcast([PB, T, K, H]),
                        op=OP.mult)
                    if DBG and li == 0 and nb == 0:
                        mt = min(T, 16)
                        mdump = sb.tile([PB, 16 * (KH + K)], f32, tag="mdump")
                        nc.gpsimd.memset(mdump[:], 0.0)
                        nc.vector.tensor_copy(
                            out=mdump[:, :mt * (KH + K)],
                            in_=M_t[:, :mt, :].rearrange("p t w -> p (t w)"))
                        nc.sync.dma_start(out=dbg["M"][:], in_=mdump[:])
                        edump = sb.tile([PB, 16 * K], f32, tag="edump")
                        nc.gpsimd.memset(edump[:], 0.0)
                        nc.vector.tensor_copy(
                            out=edump[:, :mt * K],
                            in_=er_e[:].rearrange("p t k -> p (t k)"))
                        nc.sync.dma_start(out=dbg["ere"][:], in_=edump[:])
                    # aggregate
                    U = ps.tile([PB, KH + K], f32, tag="U")
                    for t in range(T):
                        nc.tensor.matmul(U[:], lhsT=S_t[:, t * PB:(t + 1) * PB],
                                         rhs=M_t[:, t, :],
                                         start=(t == 0), stop=(t == T - 1))
                    # epilogue
                    seps = sb.tile([PB, K], f32, tag="seps")
                    nc.scalar.activation(out=seps[:], in_=U[:, KH:KH + K],
                                         func=AF.Copy, bias=1e-30)
                    invs = sb.tile([PB, K], f32, tag="invs")
                    nc.vector.reciprocal(out=invs[:], in_=seps[:])
                    xo = sb.tile([PB, KH], bft, tag="xo")
                    nc.vector.tensor_tensor(
                        out=xo[:].rearrange("p (k h) -> p k h", k=K),
                        in0=U[:, :KH].rearrange("p (k h) -> p k h", k=K),
                        in1=invs[:].unsqueeze(-1).to_broadcast([PB, K, H]),
                        op=OP.mult)
                    if li == 0:
                        res = ps2.tile([PB, KH], f32, tag="res")
                        nc.tensor.matmul(res[:],
                                         lhsT=c_t["hT"][:, nb * PB:nb * PB + PB],
                                         rhs=c_t["resW0"][:], start=True,
                                         stop=True)
                        resb = sb.tile([PB, KH], bft, tag="resb")
                        nc.scalar.activation(out=resb[:], in_=res[:],
                                             func=AF.Copy)
                        nc.vector.tensor_tensor(out=xo[:], in0=xo[:],
                                                in1=resb[:], op=OP.add)
                    else:
                        nc.vector.tensor_tensor(out=xo[:], in0=xo[:],
                                                in1=x_sh[:, nb, :], op=OP.add)
                    if use_bias[li]:
                        nc.vector.tensor_tensor(out=xo[:], in0=xo[:],
                                                in1=brep[li][:], op=OP.add)
                    if li < 2:
                        nc.scalar.activation(out=x_sh[:, nb, :], in_=xo[:],
                                             func=AF.Relu)
                        r1 = min((nb + 1) * PB, NSH) - nb * PB
                        nc.sync.dma_start(
                            out=x_hbm[li][nb * PB:nb * PB + r1, :],
                            in_=x_sh[:r1, nb, :])
                    else:
                        xr = sb.tile([PB, KH], bft, tag="xr")
                        nc.scalar.activation(out=xr[:], in_=xo[:], func=AF.Relu)
                        # mean over heads
                        m1 = sb.tile([PB, H], bft, tag="m1")
                        nc.vector.tensor_tensor(out=m1[:], in0=xr[:, 0:H],
                                                in1=xr[:, H:2 * H], op=OP.add)
                        m2 = sb.tile([PB, H], bft, tag="m2")
                        nc.vector.tensor_tensor(out=m2[:], in0=xr[:, 2 * H:3 * H],
                                                in1=xr[:, 3 * H:4 * H], op=OP.add)
                        nc.vector.tensor_tensor(out=m1[:], in0=m1[:], in1=m2[:],
                                                op=OP.add)
                        nc.scalar.activation(out=x3_sb[:, nb, :], in_=m1[:],
                                             func=AF.Copy, scale=0.25)

                if DBG:
                    if li < 2:
                        nc.sync.dma_start(
                            out=dbg[f"x{li+1}"][:],
                            in_=x_sh[:].rearrange("p b w -> p (b w)"))
                    else:
                        nc.sync.dma_start(
                            out=dbg["x3"][:],
                            in_=x3_sb[:].rearrange("p b w -> p (b w)"))

                sc_edge.__exit__(None, None, None)

            # ================= readout =================
            gsps = ps.tile([GSH, H], f32, tag="zps")
            for nb in range(NBLK):
                # w = sigmoid(x3 @ Wg + bg) via row-dot
                wx = sb.tile([PB, H], f32, tag="wx")
                nc.vector.tensor_tensor(out=wx[:], in0=x3_sb[:, nb, :],
                                        in1=c_t["Wg_rep"][:], op=OP.mult)
                wcol = sb.tile([PB, 1], f32, tag="wcol")
                nc.vector.reduce_sum(out=wcol[:], in_=wx[:],
                                     axis=mybir.AxisListType.X)
                wsig = sb.tile([PB, 1], f32, tag="wsig")
                nc.scalar.activation(out=wsig[:], in_=wcol[:], func=AF.Sigmoid,
                                     bias=meta["bg"])
                wx3 = sb.tile([PB, H], bft, tag="wx3")
                nc.vector.tensor_tensor(
                    out=wx3[:], in0=x3_sb[:, nb, :],
                    in1=wsig[:].to_broadcast([PB, H]), op=OP.mult)
                nc.tensor.matmul(gsps[:], lhsT=c_t["gsel"][:, nb * GSH:(nb + 1) * GSH],
                                 rhs=wx3[:], start=(nb == 0),
                                 stop=(nb == NBLK - 1))
                # x3 transpose for segment max
                tps = ps2.tile([H, PB], f32, tag="res")
                nc.tensor.transpose(tps[:], x3_sb[:, nb, :], c_t["ident"][:])
                nc.scalar.activation(out=x3T[:, nb * PB:(nb + 1) * PB],
                                     in_=tps[:], func=AF.Copy)
            # segment max per graph
            gmT = st.tile([H, GSH], f32, tag="gmT")
            for g, (s, e) in enumerate(granges):
                nc.vector.reduce_max(out=gmT[:, g:g + 1], in_=x3T[:, s:e],
                                     axis=mybir.AxisListType.X)
            # gfT = [gs^T ; gmT]  [2H, GSH]
            gs_sb = sb.tile([GSH, H], f32, tag="gs_sb")
            nc.vector.tensor_copy(out=gs_sb[:], in_=gsps[:])
            gsT = ps2.tile([H, GSH], f32, tag="erps")
            nc.tensor.transpose(gsT[:], gs_sb[:], c_t["ident"][:GSH, :GSH])
            gfT = sb.tile([2 * H, GSH], f32, tag="gfT")
            nc.scalar.activation(out=gfT[:H, :], in_=gsT[:], func=AF.Copy)
            nc.vector.tensor_copy(out=gfT[H:, :], in_=gmT[:])
            # y1T = relu(Wm1^T @ gfT + bm1); BN; out = sigmoid(y2T^T @ Wm2 + bm2)
            y1ps = ps.tile([H, GSH], f32, tag="zps")
            nc.tensor.matmul(y1ps[:], lhsT=c_t["Wm1"][:], rhs=gfT[:],
                             start=True, stop=True)
            y1 = sb.tile([H, GSH], f32, tag="y1")
            nc.scalar.activation(out=y1[:], in_=y1ps[:], func=AF.Relu,
                                 bias=c_t["bm1"][:])
            y2 = sb.tile([H, GSH], f32, tag="y2")
            nc.scalar.activation(out=y2[:], in_=y1[:], func=AF.Identity,
                                 scale=c_t["bn_a"][:], bias=c_t["bn_b"][:])
            ops_ = ps.tile([GSH, 1], f32, tag="U")
            nc.tensor.matmul(ops_[:], lhsT=y2[:], rhs=c_t["Wm2"][:],
                             start=True, stop=True)
            osb = sb.tile([GSH, 1], f32, tag="osb")
            nc.scalar.activation(out=osb[:], in_=ops_[:], func=AF.Sigmoid,
                                 bias=meta["bm2"])
            nc.sync.dma_start(out=out_ext[:], in_=osb[:])

    nc.compile()
    return nc


# --------------------------------------------------------------------------

def kernel(**inputs):
    from concourse.bass_utils import run_bass_kernel_spmd

    in_maps, meta = _prep(inputs)
    key = meta["N"], meta["E"], meta["G"], meta["TT"]
    if key not in _CACHE:
        _CACHE[key] = _build(meta)
    nc = _CACHE[key]
    res = run_bass_kernel_spmd(nc, in_maps, core_ids=list(range(NC)))
    out = np.concatenate([res.results[c]["out"] for c in range(NC)], axis=0)
    return out.astype(np.float32)


# revision 14
# speedup vs baseline: 1.1282x; 1.0964x over previous
"""EnhancedGAT (3-layer GAT + WeightedSumAndMax readout + MLP) on 8 TRN2
NeuronCores.

Sharding: nodes/graphs partitioned 8 ways (graph-aligned); GAT/MLP weights
replicated. Per layer: node-sharded dense projection -> AllGather of a compact
bf16 node table [z|el|er] -> edge phase with dma_gather row gathers (by src)
and selection-matrix PSUM aggregation (by dst). Selection matrices S / S_T are
host-precomputed constants (edge structure is static per call).

Self-contained: hardcodes the problem geometry; host side only reorders /
packs indices and casts weights (no FLOPs of the model are done on host).
"""
import math
import numpy as np
import ml_dtypes

bf16 = ml_dtypes.bfloat16

NC = 8           # cores
PB = 128         # partition/block size
TMAX = 8         # max tiles per dma_gather op (num_idxs <= 1024)
SPLIT = 32768    # int16 index range split for the gather table
NQ = 4           # SWDGE queues
EPS = 1e-5

_CACHE = {}


# --------------------------------------------------------------------------
# host-side preprocessing
# --------------------------------------------------------------------------

def _ceil(a, b):
    return -(-a // b)


def _pack_idx16(lst):
    """gather list -> [128, n/16] int16 tile (16-wrapped, replicated x8)."""
    n = len(lst)
    assert n % 16 == 0
    a = np.asarray(lst, np.int16).reshape(n // 16, 16).T.copy()  # [16, n/16]
    return np.tile(a, (8, 1))


def _prep(inputs):
    h = np.asarray(inputs["h"], np.float32)
    src = np.asarray(inputs["src"]).astype(np.int64).ravel()
    dst = np.asarray(inputs["dst"]).astype(np.int64).ravel()
    gid = np.asarray(inputs["graph_ids"]).astype(np.int64).ravel()

    N, F = h.shape
    E = src.shape[0]
    G = int(gid.max()) + 1
    K, H = np.asarray(inputs["al0"]).shape
    KH = K * H

    assert N % NC == 0 and G % NC == 0 and F == PB and KH == 2 * PB
    NSH = N // NC
    GSH = G // NC
    NBLK = _ceil(NSH, PB)
    XR = NBLK * PB           # padded x rows per shard
    WROW = 3 * KH // 2       # 384 bf16 = 768B table row: [z 256|el 4|er 4|pad]
    assert (WROW * 2) % 256 == 0

    # graph boundaries must align with node shards
    bounds = np.searchsorted(gid, np.arange(G))
    assert all(bounds[c * GSH] == c * NSH for c in range(NC))

    # ---- edge partition: core = dst // NSH, block = local dst // 128
    core_of = dst // NSH
    dl = dst - core_of * NSH
    blk = dl // PB
    dib = (dl % PB).astype(np.int32)          # dst-in-block column
    half = (src >= SPLIT).astype(np.int64)
    key = (core_of * NBLK + blk) * 2 + half
    order = np.argsort(key, kind="stable")
    cnt = np.bincount(key, minlength=NC * NBLK * 2).reshape(NC, NBLK, 2)
    R = _ceil(cnt, PB).max(axis=0)            # [NBLK, 2] tiles, shared SPMD

    Tb = R.sum(axis=1)                        # tiles per block
    TT = int(Tb.sum())                        # total tiles per core
    tile_base = np.zeros(NBLK, np.int64)      # first global tile of block
    tile_base[1:] = np.cumsum(Tb)[:-1]

    # gather op layout (shared): ops are (block, half) chunks of <= TMAX tiles
    ops = []                                  # (blk, gt0, ntiles, half)
    for b in range(NBLK):
        t0 = int(tile_base[b])
        for hf in range(2):
            r = int(R[b, hf])
            while r > 0:
                nt = min(r, TMAX)
                ops.append((b, t0, nt, hf))
                t0 += nt
                r -= nt
    idx_cols = sum(nt * PB // 16 for (_, _, nt, _) in ops)

    starts = np.zeros(NC * NBLK * 2 + 1, np.int64)
    starts[1:] = np.cumsum(cnt.ravel())

    per_core = []
    for c in range(NC):
        opcnt = []
        gidx = np.zeros((16, idx_cols), np.int16)
        S = np.zeros((PB, TT * PB), bf16)
        ST = np.zeros((PB, TT * PB), bf16)

        col = 0
        for (b, gt0, nt, hf) in ops:
            kk = (c * NBLK + b) * 2 + hf
            seg = order[starts[kk]:starts[kk + 1]]
            # slots covered by this op: global tiles gt0..gt0+nt
            base_slot = (gt0 - tile_base[b] - (R[b, 0] if hf else 0)) * PB
            lo_e = base_slot
            hi_e = min(base_slot + nt * PB, len(seg))
            rows = np.full(nt * PB, -1, np.int64)
            if hi_e > lo_e:
                ee = seg[lo_e:hi_e]
                rows[:hi_e - lo_e] = src[ee] - (SPLIT if hf else 0)
                # slot i in op -> tile gt0 + i//128, partition i%128
                i = np.arange(hi_e - lo_e)
                p_slot = i % PB
                t_slot = gt0 + i // PB
                S[p_slot, t_slot * PB + dib[ee]] = 1.0
                ST[dib[ee], t_slot * PB + p_slot] = 1.0
            ncol = nt * PB // 16
            gidx[:, col:col + ncol] = rows.reshape(ncol, 16).T
            col += ncol
            opcnt.append(max(hi_e - lo_e, 0))
        gidx = np.tile(gidx, (8, 1))

        # graph selector [128, NBLK*GSH] f32 and graph node ranges
        gsel = np.zeros((PB, NBLK * GSH), bf16)
        nloc = np.arange(NSH)
        gloc = gid[c * NSH:(c + 1) * NSH] - c * GSH
        gsel[nloc % PB, (nloc // PB) * GSH + gloc] = 1.0
        granges = [(int(bounds[c * GSH + g] - c * NSH),
                    int((bounds[c * GSH + g + 1] if c * GSH + g + 1 < G else N)
                        - c * NSH))
                   for g in range(GSH)]

        hT = np.zeros((PB, XR), bf16)
        hT[:, :NSH] = h[c * NSH:(c + 1) * NSH].T.astype(bf16)

        per_core.append(dict(gidx=gidx, S=S, ST=ST, gsel=gsel, hT=hT,
                             opcnt=np.asarray(opcnt, np.int32).reshape(1, -1),
                             granges=granges))

    # ---- weights
    def aug(W, al, ar):
        W = np.asarray(W, np.float32)
        al = np.asarray(al, np.float32)
        ar = np.asarray(ar, np.float32)
        A = np.zeros((KH, 2 * K), np.float32)
        for k in range(K):
            A[k * H:(k + 1) * H, k] = al[k]
            A[k * H:(k + 1) * H, K + k] = ar[k]
        return np.concatenate([W, W @ A], axis=1).astype(bf16)  # [F, KH+2K]

    wd = dict(
        W0a=aug(inputs["W0"], inputs["al0"], inputs["ar0"]),
        W1a=aug(inputs["W1"], inputs["al1"], inputs["ar1"]),
        W2a=aug(inputs["W2"], inputs["al2"], inputs["ar2"]),
        resW0=np.asarray(inputs["resW0"], np.float32).astype(bf16),
        Wg_rep=np.tile(np.asarray(inputs["Wg"], np.float32).reshape(1, H),
                       (PB, 1)),
        Wm1=np.asarray(inputs["Wm1"], np.float32),            # [2H, H]
        Wm2=np.asarray(inputs["Wm2"], np.float32),            # [H, 1]
        bm1=np.asarray(inputs["bm1"], np.float32).reshape(H, 1),
        ident=np.eye(PB, dtype=np.float32),
    )
    bn_a = (np.asarray(inputs["bn_g"], np.float32)
            / np.sqrt(np.asarray(inputs["bn_v"], np.float32) + EPS))
    wd["bn_a"] = bn_a.reshape(H, 1)
    wd["bn_b"] = (np.asarray(inputs["bn_b"], np.float32)
                  - np.asarray(inputs["bn_m"], np.float32) * bn_a).reshape(H, 1)
    biases = [np.asarray(inputs[k], np.float32) for k in ("b0", "b1", "b2")]
    use_bias = [bool(np.any(b != 0)) for b in biases]
    for li in range(3):
        if use_bias[li]:
            wd[f"brep{li}"] = np.tile(biases[li].reshape(1, KH), (PB, 1))

    meta = dict(N=N, F=F, E=E, G=G, K=K, H=H, KH=KH, NSH=NSH, GSH=GSH,
                NBLK=NBLK, XR=XR, WROW=WROW, TT=TT, R=R, Tb=Tb,
                tile_base=tile_base, ops=ops, idx_cols=idx_cols,
                use_bias=use_bias,
                bg=float(np.asarray(inputs["bg"]).ravel()[0]),
                bm2=float(np.asarray(inputs["bm2"]).ravel()[0]),
                granges=per_core[0]["granges"])

    in_maps = []
    for c in range(NC):
        m = dict(per_core[c])
        gr = m.pop("granges")
        m.update(wd)
        in_maps.append(m)
    for pc in per_core[1:]:
        assert pc["granges"] == per_core[0]["granges"]
    return in_maps, meta


# --------------------------------------------------------------------------
# device program
# --------------------------------------------------------------------------

def _build(meta):
    from concourse import bass, bacc, tile, mybir

    f32 = mybir.dt.float32
    bft = mybir.dt.bfloat16
    i16 = mybir.dt.int16
    AF = mybir.ActivationFunctionType
    OP = mybir.AluOpType

    N, F, KH, K, H = meta["N"], meta["F"], meta["KH"], meta["K"], meta["H"]
    NSH, GSH, NBLK, XR = meta["NSH"], meta["GSH"], meta["NBLK"], meta["XR"]
    WROW, TT = meta["WROW"], meta["TT"]
    R, Tb, tile_base, ops = meta["R"], meta["Tb"], meta["tile_base"], meta["ops"]
    use_bias = meta["use_bias"]
    granges = meta["granges"]
    NW = KH + 2 * K                      # 264 dense out cols

    nc = bacc.Bacc("TRN2", target_bir_lowering=False, debug=False,
                   enable_asserts=False, num_devices=NC, num_swdge_queues=NQ)

    # ---- I/O
    ein = {}
    for name, shape, dt in [
        ("gidx", [PB, meta["idx_cols"]], i16),
        ("opcnt", [1, len(meta["ops"])], mybir.dt.int32),
        ("S", [PB, TT * PB], bft),
        ("ST", [PB, TT * PB], bft),
        ("gsel", [PB, NBLK * GSH], bft),
        ("hT", [PB, XR], bft),
        ("W0a", [F, NW], bft),
        ("W1a", [KH, NW], bft),
        ("W2a", [KH, NW], bft),
        ("resW0", [F, KH], bft),
        ("Wg_rep", [PB, H], f32),
        ("Wm1", [2 * H, H], f32),
        ("Wm2", [H, 1], f32),
        ("bm1", [H, 1], f32),
        ("bn_a", [H, 1], f32),
        ("bn_b", [H, 1], f32),
        ("ident", [PB, PB], f32),
    ]:
        ein[name] = nc.dram_tensor(name, shape, dt, kind="ExternalInput")
    for li in range(3):
        if use_bias[li]:
            ein[f"brep{li}"] = nc.dram_tensor(f"brep{li}", [PB, KH], f32,
                                              kind="ExternalInput")
    out_ext = nc.dram_tensor("out", [GSH, 1], f32, kind="ExternalOutput")
    dbg = {}
    import os as _os
    DBG = bool(int(_os.environ.get("KGAT_DEBUG", "0")))
    if DBG:
        for li in range(3):
            dbg[f"x{li+1}"] = nc.dram_tensor(
                f"dbg_x{li+1}", [PB, NBLK * (KH if li < 2 else H)], f32,
                kind="ExternalOutput")
        dbg["tab"] = nc.dram_tensor("dbg_tab", [N, WROW], bft,
                                    kind="ExternalOutput")
        dbg["M"] = nc.dram_tensor("dbg_M", [PB, 16 * (KH + K)], f32,
                                  kind="ExternalOutput")
        dbg["ere"] = nc.dram_tensor("dbg_ere", [PB, 16 * K], f32,
                                    kind="ExternalOutput")

    # ---- internal DRAM
    cc_in = [nc.dram_tensor(f"cc_in{li}", [NSH, WROW], bft, kind="Internal")
             for li in range(3)]
    tables = [nc.dram_tensor(f"table{li}", [N, WROW], bft, kind="Internal",
                             addr_space="Shared") for li in range(3)]
    x_hbm = [nc.dram_tensor(f"x_hbm{li}", [XR, KH], bft, kind="Internal")
             for li in range(2)]

    qctr = [0]

    def next_q():
        q = qctr[0] % NQ
        qctr[0] += 1
        return q

    with tile.TileContext(nc) as tc:
        with (
            tc.tile_pool(name="const", bufs=1) as cp,
            tc.tile_pool(name="state", bufs=1) as st,
            tc.tile_pool(name="sb", bufs=3) as sb,
            tc.tile_pool(name="gbuf", bufs=3) as gb,
            tc.tile_pool(name="xp", bufs=1) as xp,
            tc.tile_pool(name="ps", bufs=2, space="PSUM") as ps,
            tc.tile_pool(name="ps2", bufs=2, space="PSUM") as ps2,
        ):
            # ---- constants to SBUF
            c_t = {}
            for name in ["gsel", "hT", "resW0", "Wg_rep",
                         "Wm1", "Wm2", "bm1", "bn_a", "bn_b", "ident"]:
                tsr = ein[name]
                t = cp.tile(list(tsr.shape), tsr.dtype, tag=f"c_{name}")
                nc.sync.dma_start(out=t[:], in_=tsr[:])
                c_t[name] = t
            W_chunks = {}
            for name in ["W0a", "W1a", "W2a"]:
                tsr = ein[name]
                nch = tsr.shape[0] // PB
                lst = []
                for kc in range(nch):
                    t = cp.tile([PB, NW], bft, tag=f"c_{name}_{kc}")
                    nc.sync.dma_start(out=t[:],
                                      in_=tsr[kc * PB:(kc + 1) * PB, :])
                    lst.append(t)
                W_chunks[name] = lst
            brep = {}
            for li in range(3):
                if use_bias[li]:
                    t = cp.tile([PB, KH], f32, tag=f"c_brep{li}")
                    nc.sync.dma_start(out=t[:], in_=ein[f"brep{li}"][:])
                    brep[li] = t
            gidx_sb = cp.tile([PB, meta["idx_cols"]], i16, tag="c_gidx")
            nc.sync.dma_start(out=gidx_sb[:], in_=ein["gidx"][:])
            opcnt_sb = cp.tile([1, len(meta["ops"])], mybir.dt.int32,
                               tag="c_opcnt")
            nc.sync.dma_start(out=opcnt_sb[:], in_=ein["opcnt"][:])
            cnt_reg = nc.gpsimd.alloc_register("gather_cnt")

            # ---- persistent state
            x_sh = st.tile([PB, NBLK, KH], bft, tag="x_sh")
            er_sh = st.tile([PB, NBLK, 2 * K], bft, tag="er_sh")
            x3_sb = st.tile([PB, NBLK, H], f32, tag="x3")
            x3T = st.tile([H, XR], f32, tag="x3T")
            zero128 = cp.tile([PB, KH], bft, tag="zeros")
            nc.gpsimd.memset(zero128[:], 0.0)
            # zero the x_hbm pad rows once
            for li in range(2):
                if XR > NSH:
                    nc.sync.dma_start(out=x_hbm[li][NSH:XR, :],
                                      in_=zero128[:XR - NSH, :])

            TMAXT = int(Tb.max()) if len(Tb) else 1
            for _gi in range(3):
                gz = gb.tile([PB, TMAXT, WROW], bft, tag="G")
                nc.gpsimd.memset(gz[:], 0.0)
            op_col = {}
            col = 0
            for oi, (b, gt0, nt, hf) in enumerate(ops):
                op_col[oi] = col
                col += nt * PB // 16

            # ================= layers =================
            for li in range(3):
                Wa = W_chunks[["W0a", "W1a", "W2a"][li]]
                tab = tables[li]
                cci = cc_in[li]

                # ---- dense: z shard -> cc_in
                sc_dense = nc.named_scope(f"L{li}_dense"); sc_dense.__enter__()
                if li == 0:
                    xT_chunks = [c_t["hT"]]
                else:
                    xT_chunks = []
                    for kc in range(2):
                        t = xp.tile([PB, XR], bft, tag=f"xT{kc}")
                        nc.sync.dma_start(
                            out=t[:], in_=x_hbm[li - 1][:, kc * PB:(kc + 1) * PB],
                            transpose=True)
                        xT_chunks.append(t)
                for nb in range(NBLK):
                    r1 = min((nb + 1) * PB, NSH) - nb * PB
                    zps = ps.tile([PB, NW], f32, tag="zps")
                    for kc, xT in enumerate(xT_chunks):
                        nc.tensor.matmul(
                            zps[:], lhsT=xT[:, nb * PB:nb * PB + PB],
                            rhs=Wa[kc][:],
                            start=(kc == 0), stop=(kc == len(xT_chunks) - 1))
                    zsb = sb.tile([PB, NW], bft, tag="zsb")
                    nc.vector.tensor_copy(out=zsb[:], in_=zps[:])
                    nc.vector.tensor_copy(out=er_sh[:, nb, :],
                                          in_=zps[:, KH:KH + 2 * K])
                    nc.sync.dma_start(out=cci[nb * PB:nb * PB + r1, :NW],
                                      in_=zsb[:r1, :])
                sc_dense.__exit__(None, None, None)
                # ---- exchange
                sc_ag = nc.named_scope(f"L{li}_ag"); sc_ag.__enter__()
                nc.gpsimd.collective_compute(
                    "AllGather", OP.bypass,
                    replica_groups=[list(range(NC))],
                    ins=[cci[:]], outs=[tab[:]],
                )

                if DBG and li == 0:
                    nc.sync.dma_start(out=dbg["tab"][:], in_=tab[:])
                sc_ag.__exit__(None, None, None)
                # ---- edge phase
                sc_edge = nc.named_scope(f"L{li}_edge"); sc_edge.__enter__()
                for nb in range(NBLK):
                    T = int(Tb[nb])
                    if T == 0:
                        continue
                    t0 = int(tile_base[nb])
                    G_t = gb.tile([PB, T, WROW], bft, tag="G")
                    # gathers (ops of this block)
                    for oi, (b, gt0, nt, hf) in enumerate(ops):
                        if b != nb:
                            continue
                        lt = gt0 - t0
                        src_ap = tab[SPLIT:, :] if hf else tab[:, :]
                        nc.gpsimd.reg_load(cnt_reg, opcnt_sb[0:1, oi:oi + 1])
                        nc.gpsimd.dma_gather(
                            G_t[:, lt:lt + nt, :], src_ap,
                            gidx_sb[:, op_col[oi]:op_col[oi] + nt * PB // 16],
                            num_idxs=nt * PB, num_idxs_reg=cnt_reg,
                            elem_size=WROW, queue_num=next_q(),
                        )
                    S_t = gb.tile([PB, T * PB], bft, tag="S")
                    ST_t = gb.tile([PB, T * PB], bft, tag="ST")
                    nc.sync.dma_start(
                        out=S_t[:], in_=ein["S"][:, t0 * PB:(t0 + T) * PB])
                    nc.scalar.dma_start(
                        out=ST_t[:], in_=ein["ST"][:, t0 * PB:(t0 + T) * PB])

                    # er broadcast to edges: erps[:, t, :] = ST_t.T @ er_blk
                    erps = ps2.tile([PB, T, K], f32, tag="erps")
                    for t in range(T):
                        nc.tensor.matmul(
                            erps[:, t, :], lhsT=ST_t[:, t * PB:(t + 1) * PB],
                            rhs=er_sh[:, nb, K:2 * K], start=True, stop=True)
                    er_e = sb.tile([PB, T, K], bft, tag="er_e")
                    nc.scalar.activation(out=er_e[:], in_=erps[:], func=AF.Copy)
                    # e_pre = el[src] + er[dst]
                    epre = sb.tile([PB, T, K], bft, tag="epre")
                    nc.vector.tensor_tensor(
                        out=epre[:], in0=G_t[:, :, KH:KH + K], in1=er_e[:],
                        op=OP.add)
                    # w = exp(leaky_relu(e_pre))
                    lr2 = sb.tile([PB, T, K], bft, tag="lr2")
                    nc.vector.tensor_scalar(out=lr2[:], in0=epre[:],
                                            scalar1=0.2, scalar2=None,
                                            op0=OP.mult)
                    lr = sb.tile([PB, T, K], bft, tag="lr")
                    nc.vector.tensor_tensor(out=lr[:], in0=epre[:], in1=lr2[:],
                                            op=OP.max)
                    M_t = gb.tile([PB, T, KH + K], bft, tag="M")
                    nc.scalar.activation(out=M_t[:, :, KH:KH + K], in_=lr[:],
                                         func=AF.Exp)
                    # messages = w (bcast per head) * z
                    nc.vector.tensor_tensor(
                        out=M_t[:, :, :KH].rearrange("p t (k h) -> p t k h", k=K),
                        in0=G_t[:, :, :KH].rearrange("p t (k h) -> p t k h", k=K),
                        in1=M_t[:, :, KH:KH + K].unsqueeze(-1)
                            .to_broadcast([PB, T, K, H]),
                        op=OP.mult)
                    if DBG and li == 0 and nb == 0:
                        mt = min(T, 16)
                        mdump = sb.tile([PB, 16 * (KH + K)], f32, tag="mdump")
                        nc.gpsimd.memset(mdump[:], 0.0)
                        nc.vector.tensor_copy(
                            out=mdump[:, :mt * (KH + K)],
                            in_=M_t[:, :mt, :].rearrange("p t w -> p (t w)"))
                        nc.sync.dma_start(out=dbg["M"][:], in_=mdump[:])
                        edump = sb.tile([PB, 16 * K], f32, tag="edump")
                        nc.gpsimd.memset(edump[:], 0.0)
                        nc.vector.tensor_copy(
                            out=edump[:, :mt * K],
                            in_=er_e[:].rearrange("p t k -> p (t k)"))
                        nc.sync.dma_start(out=dbg["ere"][:], in_=edump[:])
                    # aggregate
                    U = ps.tile([PB, KH + K], f32, tag="U")
                    for t in range(T):
                        nc.tensor.matmul(U[:], lhsT=S_t[:, t * PB:(t + 1) * PB],
                                         rhs=M_t[:, t, :],
                                         start=(t == 0), stop=(t == T - 1))
                    # epilogue
                    seps = sb.tile([PB, K], f32, tag="seps")
                    nc.scalar.activation(out=seps[:], in_=U[:, KH:KH + K],
                                         func=AF.Copy, bias=1e-30)
                    invs = sb.tile([PB, K], f32, tag="invs")
                    nc.vector.reciprocal(out=invs[:], in_=seps[:])
                    xo = sb.tile([PB, KH], bft, tag="xo")
                    nc.vector.tensor_tensor(
                        out=xo[:].rearrange("p (k h) -> p k h", k=K),
                        in0=U[:, :KH].rearrange("p (k h) -> p k h", k=K),
                        in1=invs[:].unsqueeze(-1).to_broadcast([PB, K, H]),
                        op=OP.mult)
                    if li == 0:
                        res = ps2.tile([PB, KH], f32, tag="res")
                        nc.tensor.matmul(res[:],
                                         lhsT=c_t["hT"][:, nb * PB:nb * PB + PB],
                                         rhs=c_t["resW0"][:], start=True,
                                         stop=True)
                        resb = sb.tile([PB, KH], bft, tag="resb")
                        nc.scalar.activation(out=resb[:], in_=res[:],
                                             func=AF.Copy)
                        nc.vector.tensor_tensor(out=xo[:], in0=xo[:],
                                                in1=resb[:], op=OP.add)
                    else:
                        nc.vector.tensor_tensor(out=xo[:], in0=xo[:],
                                                in1=x_sh[:, nb, :], op=OP.add)
                    if use_bias[li]:
                        nc.vector.tensor_tensor(out=xo[:], in0=xo[:],
                                                in1=brep[li][:], op=OP.add)
                    if li < 2:
                        nc.scalar.activation(out=x_sh[:, nb, :], in_=xo[:],
                                             func=AF.Relu)
                        r1 = min((nb + 1) * PB, NSH) - nb * PB
                        nc.sync.dma_start(
                            out=x_hbm[li][nb * PB:nb * PB + r1, :],
                            in_=x_sh[:r1, nb, :])
                    else:
                        xr = sb.tile([PB, KH], bft, tag="xr")
                        nc.scalar.activation(out=xr[:], in_=xo[:], func=AF.Relu)
                        # mean over heads
                        m1 = sb.tile([PB, H], bft, tag="m1")
                        nc.vector.tensor_tensor(out=m1[:], in0=xr[:, 0:H],
                                                in1=xr[:, H:2 * H], op=OP.add)
                        m2 = sb.tile([PB, H], bft, tag="m2")
                        nc.vector.tensor_tensor(out=m2[:], in0=xr[:, 2 * H:3 * H],
                                                in1=xr[:, 3 * H:4 * H], op=OP.add)
                        nc.vector.tensor_tensor(out=m1[:], in0=m1[:], in1=m2[:],
                                                op=OP.add)
                        nc.scalar.activation(out=x3_sb[:, nb, :], in_=m1[:],
                                             func=AF.Copy, scale=0.25)

                if DBG:
                    if li < 2:
                        nc.sync.dma_start(
                            out=dbg[f"x{li+1}"][:],
                            in_=x_sh[:].rearrange("p b w -> p (b w)"))
                    else:
                        nc.sync.dma_start(
                            out=dbg["x3"][:],
                            in_=x3_sb[:].rearrange("p b w -> p (b w)"))

                sc_edge.__exit__(None, None, None)

            # ================= readout =================
            gsps = ps.tile([GSH, H], f32, tag="zps")
            for nb in range(NBLK):
                # w = sigmoid(x3 @ Wg + bg) via row-dot
                wx = sb.tile([PB, H], f32, tag="wx")
                nc.vector.tensor_tensor(out=wx[:], in0=x3_sb[:, nb, :],
                                        in1=c_t["Wg_rep"][:], op=OP.mult)
                wcol = sb.tile([PB, 1], f32, tag="wcol")
                nc.vector.reduce_sum(out=wcol[:], in_=wx[:],
                                     axis=mybir.AxisListType.X)
                wsig = sb.tile([PB, 1], f32, tag="wsig")
                nc.scalar.activation(out=wsig[:], in_=wcol[:], func=AF.Sigmoid,
                                     bias=meta["bg"])
                wx3 = sb.tile([PB, H], bft, tag="wx3")
                nc.vector.tensor_tensor(
                    out=wx3[:], in0=x3_sb[:, nb, :],
                    in1=wsig[:].to_broadcast([PB, H]), op=OP.mult)
                nc.tensor.matmul(gsps[:], lhsT=c_t["gsel"][:, nb * GSH:(nb + 1) * GSH],
                                 rhs=wx3[:], start=(nb == 0),
                                 stop=(nb == NBLK - 1))
                # x3 transpose for segment max
                tps = ps2.tile([H, PB], f32, tag="res")
                nc.tensor.transpose(tps[:], x3_sb[:, nb, :], c_t["ident"][:])
                nc.scalar.activation(out=x3T[:, nb * PB:(nb + 1) * PB],
                                     in_=tps[:], func=AF.Copy)
            # segment max per graph
            gmT = st.tile([H, GSH], f32, tag="gmT")
            for g, (s, e) in enumerate(granges):
                nc.vector.reduce_max(out=gmT[:, g:g + 1], in_=x3T[:, s:e],
                                     axis=mybir.AxisListType.X)
            # gfT = [gs^T ; gmT]  [2H, GSH]
            gs_sb = sb.tile([GSH, H], f32, tag="gs_sb")
            nc.vector.tensor_copy(out=gs_sb[:], in_=gsps[:])
            gsT = ps2.tile([H, GSH], f32, tag="erps")
            nc.tensor.transpose(gsT[:], gs_sb[:], c_t["ident"][:GSH, :GSH])
            gfT = sb.tile([2 * H, GSH], f32, tag="gfT")
            nc.scalar.activation(out=gfT[:H, :], in_=gsT[:], func=AF.Copy)
            nc.vector.tensor_copy(out=gfT[H:, :], in_=gmT[:])
            # y1T = relu(Wm1^T @ gfT + bm1); BN; out = sigmoid(y2T^T @ Wm2 + bm2)
            y1ps = ps.tile([H, GSH], f32, tag="zps")
            nc.tensor.matmul(y1ps[:], lhsT=c_t["Wm1"][:], rhs=gfT[:],
                             start=True, stop=True)
            y1 = sb.tile([H, GSH], f32, tag="y1")
            nc.scalar.activation(out=y1[:], in_=y1ps[:], func=AF.Relu,
                                 bias=c_t["bm1"][:])
            y2 = sb.tile([H, GSH], f32, tag="y2")
            nc.scalar.activation(out=y2[:], in_=y1[:], func=AF.Identity,
                                 scale=c_t["bn_a"][:], bias=c_t["bn_b"][:])
            ops_ = ps.tile([GSH, 1], f32, tag="U")
            nc.tensor.matmul(ops_[:], lhsT=y2[:], rhs=c_t["Wm2"][:],
                             start=True, stop=True)
            osb = sb.tile([GSH, 1], f32, tag="osb")
            nc.scalar.activation(out=osb[:], in_=ops_[:], func=AF.Sigmoid,
                                 bias=meta["bm2"])
            nc.sync.dma_start(out=out_ext[:], in_=osb[:])

    nc.compile()
    return nc


# --------------------------------------------------------------------------

def kernel(**inputs):
    from concourse.bass_utils import run_bass_kernel_spmd

    in_maps, meta = _prep(inputs)
    key = meta["N"], meta["E"], meta["G"], meta["TT"]
    if key not in _CACHE:
        _CACHE[key] = _build(meta)
    nc = _CACHE[key]
    res = run_bass_kernel_spmd(nc, in_maps, core_ids=list(range(NC)))
    out = np.concatenate([res.results[c]["out"] for c in range(NC)], axis=0)
    return out.astype(np.float32)
